# revision 17
# baseline (speedup 1.0000x reference)
"""Trainium2 Bass kernel for nn_CrossProduct (factorization-machine cross term).

out_b = 0.5 * [ sum_k (x_b @ v_k)^2  -  sum_i w_i x_bi^2 ],  w_i = sum_k v_ik^2

Host-side rescaling removes all per-feature weights from the device:
  x'  = x * sqrt(w/2)          (shipped fp16, feature-on-partition, chunk-major)
  v'' = v / sqrt(w)            (replicated fp16)
  => psA[k,b] = x'_b @ v''_k = (x v_k)/sqrt(2);  sq = psA^2 = (xv)^2/2
     term2_b  = sum_i x'_bi^2 = 0.5 sum_i w_i x_bi^2  (constant -1 PE weights)
  out_b = (ones64 . sq) - term2_b   accumulated in one PSUM row.

Device program per core (2048 rows, 8 contraction chunks of 128):
  - 8 full-chunk DMAs alternating two rings (sync / gpsimd queues); the
    pa phase is DMA-paced (~12us for 4.2MB at ~350GB/s).
  - pa: 32 matmuls [64,512] fp16 accumulating psA (banks 0-3).
  - x'^2 in fp8e4: DVE chunks 0-3, ACT 4-6, chunk 7 split in halves
    across both so the last term2 matmul is unblocked soon after its DMA.
  - po: term2 via fp8 DoubleRow matmuls (2 chunks per pass, halves the
    streamed rows) with constant -1 fp8 weights (memset, no DMA).
  - tail: psA squares ACT(q0,q1) in parallel with DVE copy+square (q2,q3),
    ones64 matmuls add term1 into psO row 0, copies, single 4KB DMA out.
"""

import math
from contextlib import ExitStack

import ml_dtypes
import numpy as np

import concourse.bass as bass
import concourse.bacc as bacc
import concourse.mybir as mybir
import concourse.tile as tile
from concourse.bass_utils import run_bass_kernel_spmd

F16 = mybir.dt.float16
F32 = mybir.dt.float32
F8 = mybir.dt.float8e4

N_CORES = 8
B, XD, KD = 16384, 1024, 64
BS = B // N_CORES   # 2048 batch rows per core
C = XD // 128       # 8 contraction chunks of 128

DVE_SQ = (0, 1, 2, 3)      # chunks squared on DVE (fp8 out)
ACT_SQ = (4, 5, 6)         # chunks squared on ACT
# chunk 7 (last to arrive) is squared in halves on DVE+ACT concurrently


def _body(ctx, tc, OUT, X, VW):
    nc = tc.nc
    const = ctx.enter_context(tc.tile_pool(name="const", bufs=1))
    xpool = ctx.enter_context(tc.tile_pool(name="xp", bufs=1))
    x2pool = ctx.enter_context(tc.tile_pool(name="x2p", bufs=1))
    sqpool = ctx.enter_context(tc.tile_pool(name="sqp", bufs=1))
    opool = ctx.enter_context(tc.tile_pool(name="op", bufs=1))
    psa = ctx.enter_context(tc.tile_pool(name="psA", bufs=1, space="PSUM"))
    pso = ctx.enter_context(tc.tile_pool(name="psO", bufs=1, space="PSUM"))

    # vw cols: [c*64:(c+1)*64] = v''_c; col 512 = +1 (term1 reduce weights)
    vw = const.tile([128, C * KD + 1], F16)
    nc.scalar.dma_start(vw[:], VW)
    # fp8 -1 weights for the DoubleRow term2 matmuls: [128, 2 ktiles, 64
    # cols] (dual-fp8 LDWEIGHTS needs a half-width tile; only col 0 = -1
    # matters, the rest are 0).
    vw8 = const.tile([128, 2, 64], F8)
    nc.gpsimd.memset(vw8[:], 0.0)
    nc.gpsimd.memset(vw8[:, :, 0:1], -1.0)

    xt = xpool.tile([128, C, BS], F16)
    # chunk 0 in halves on both rings so the first pa matmul starts early
    nc.sync.dma_start(xt[:, 0, 0:1024], X[0, :, 0:1024])
    nc.gpsimd.dma_start(xt[:, 0, 1024:BS], X[0, :, 1024:BS])
    for c in range(1, C):
        (nc.sync if c % 2 == 0 else nc.gpsimd).dma_start(xt[:, c], X[c])

    # Every chunk's square is split DVE||ACT so per-chunk x'^2 latency
    # (~1.15us) keeps up with the ~1.5us/chunk DMA pace and term2 matmuls
    # can fill the PE's DMA-wait gaps instead of bunching at the end.
    x2 = x2pool.tile([128, C, BS], F8)
    for c in range(C):
        nc.vector.tensor_mul(
            x2[:, c, 0:1024], xt[:, c, 0:1024], xt[:, c, 0:1024]
        )
        nc.scalar.activation(
            x2[:, c, 1024:BS], xt[:, c, 1024:BS],
            mybir.ActivationFunctionType.Square,
        )

    pa = psa.tile([64, BS], F32)
    po = pso.tile([64, BS], F32)

    def pa_mm(c):
        for q in range(4):
            nc.tensor.matmul(
                pa[:, q * 512 : (q + 1) * 512],
                vw[:, c * KD : (c + 1) * KD],
                xt[:, c, q * 512 : (q + 1) * 512],
                start=(c == 0),
                stop=(c == C - 1),
                tile_position=(0, 0),
            )

    def po_mm(p):
        # fp8 DoubleRow: contracts chunks (2p, 2p+1) in one pass
        for q in range(4):
            nc.tensor.matmul(
                po[0:64, q * 512 : (q + 1) * 512],
                vw8[:],
                x2[:, 2 * p : 2 * p + 2, q * 512 : (q + 1) * 512],
                start=(p == 0),
                stop=False,
                perf_mode=mybir.MatmulPerfMode.DoubleRow,
                tile_position=(0, 0),
            )

    pa_mm(0)
    pa_mm(1)
    pa_mm(2)
    po_mm(0)
    pa_mm(3)
    pa_mm(4)
    po_mm(1)
    pa_mm(5)
    pa_mm(6)
    po_mm(2)
    pa_mm(7)
    po_mm(3)

    # term1: square psA -> sq fp16; ACT does q0,q1 directly while DVE
    # copies+squares q2,q3 (DVE cannot read two PSUM operands).
    sq = sqpool.tile([64, BS], F16)
    sqc = sqpool.tile([64, 1024], F16)  # DVE psum-copy staging
    for q in (0, 1):
        s = slice(q * 512, (q + 1) * 512)
        nc.scalar.activation(
            sq[:, s], pa[:, s], mybir.ActivationFunctionType.Square
        )
    for q in (2, 3):
        s = slice(q * 512, (q + 1) * 512)
        t = slice((q - 2) * 512, (q - 1) * 512)
        nc.vector.tensor_scalar_mul(sqc[:, t], pa[:, s], 1.0)
        nc.vector.tensor_mul(sq[:, s], sqc[:, t], sqc[:, t])

    ones64 = vw[0:64, C * KD : C * KD + 1]
    for q in range(4):
        s = slice(q * 512, (q + 1) * 512)
        nc.tensor.matmul(
            po[0:1, s], ones64, sq[:, s],
            start=False, stop=True, tile_position=(0, 0),
        )

    outs = opool.tile([1, BS], F16)
    for q in range(4):
        s = slice(q * 512, (q + 1) * 512)
        if q % 2 == 0:
            nc.scalar.copy(outs[0:1, s], po[0:1, s])
        else:
            nc.vector.tensor_scalar_mul(outs[0:1, s], po[0:1, s], 1.0)
    nc.sync.dma_start(OUT, outs[0:1, :])


_NC_CACHE = None


def build_nc():
    global _NC_CACHE
    if _NC_CACHE is not None:
        return _NC_CACHE
    nc = bacc.Bacc("TRN2", target_bir_lowering=False, debug=False)
    X = nc.dram_tensor("X", [C, 128, BS], F16, kind="ExternalInput").ap()
    VW = nc.dram_tensor("VW", [128, C * KD + 1], F16, kind="ExternalInput").ap()
    OUT = nc.dram_tensor("OUT", [1, BS], F16, kind="ExternalOutput").ap()
    with tile.TileContext(nc) as tc:
        with ExitStack() as ctx:
            _body(ctx, tc, OUT, X, VW)
    nc.compile()
    _NC_CACHE = nc
    return nc


def make_in_maps(x, vparam):
    x = np.ascontiguousarray(x, dtype=np.float32)
    v = np.ascontiguousarray(vparam, dtype=np.float32)

    w = (v.astype(np.float64) ** 2).sum(axis=1)          # (1024,)
    w = np.maximum(w, 1e-12)
    s = np.sqrt(w / 2.0)                                 # x scale
    vn = (v / np.sqrt(w)[:, None]).astype(np.float32)    # (1024, 64)

    VWh = np.empty((128, C * KD + 1), dtype=np.float16)
    VWh[:, 0 : C * KD] = (
        vn.reshape(C, 128, KD).transpose(1, 0, 2).reshape(128, C * KD)
    )
    VWh[:, C * KD] = 1.0

    xs_all = (x * s[None, :]).astype(np.float16)         # (B, 1024)

    in_maps = []
    for i in range(N_CORES):
        xs = xs_all[i * BS : (i + 1) * BS]               # (2048, 1024)
        # X[c, p, b] = xs.T[c*128+p, b]
        A = np.ascontiguousarray(xs.T).reshape(C, 128, BS)
        in_maps.append({"X": A, "VW": VWh})
    return in_maps


LAST_RESULTS = None  # stashed BassKernelResults (for test harness profiling)
TRACE = False


def kernel(x, vparam):
    global LAST_RESULTS
    nc = build_nc()
    in_maps = make_in_maps(x, vparam)
    res = run_bass_kernel_spmd(nc, in_maps, list(range(N_CORES)), trace=TRACE)
    LAST_RESULTS = res
    out = np.concatenate(
        [
            res.results[i]["OUT"].astype(np.float32).reshape(BS, 1)
            for i in range(N_CORES)
        ],
        axis=0,
    )
    return out.astype(np.float32)
